# revision 1
# baseline (speedup 1.0000x reference)
"""Trainium2 Bass kernel for DetectionLoss (focal cls + DFL box loss).

Strategy
--------
Data-parallel over the batch: 16 images -> 8 cores x 2 images.

The reference loss only reads the feature maps at 50 target locations per
image (each target contributes only at its own FPN layer, because the layer
mask zeroes the other two layers).  Per core we:

  1. Stream the core's full feature-map shard (9.7 MB) into SBUF with large
     contiguous DMAs, split across both HWDGE queues (sync + scalar).
     Channels 0..128 land in a [128, 16800] tile; the remaining 16 channels
     are folded to full partition width as [128, 2100] by splitting each
     (layer, image) spatial block into 8 sub-blocks (partition = (c-128)*8+u).
  2. Compute, on device, the flat gather index of each (padded) target from
     the raw `targets` tensor: fx = floor(cx*W), fy = floor(cy*H),
     s = block base + fy*W + fx, plus the sub-block split (u, r) for the
     remainder tile.
  3. ap_gather (GPSIMD) the feature columns of all 128 padded targets:
     G1[c, t] = Fa[c, s_t], G2[(c,u), t] = Fb[(c,u), r_t].
  4. PE-transpose to T[t, c]; the remainder channels additionally need a
     select over u (one-hot multiply + reduce).
  5. Focal loss over the 80 class channels + DFL loss over the 4x16 bin
     channels, all on [128, <=144] tiles (DVE/ACT).
  6. Reduce the 128 per-target contributions with a ones-matmul -> [2]
     scalars (cls_sum, box_sum) per core; host sums the 8 partials.

Targets are padded host-side from 50 -> 64 per image with rows whose layer
field is 3 (matches no layer -> masked out; pure padding, no host compute).
"""

import numpy as np

import concourse.bass as bass
import concourse.mybir as mybir
import concourse.tile as tile
from concourse import bacc
from concourse.bass_utils import run_bass_kernel_spmd
from concourse.tile_rust import add_dep_helper

F32 = mybir.dt.float32
I32 = mybir.dt.int32
I16 = mybir.dt.int16
ALU = mybir.AluOpType
ACT = mybir.ActivationFunctionType
AX = mybir.AxisListType

N_CORES = 8
B = 16
BPC = B // N_CORES  # images per core
N_TGT = 50
NT_PAD = 64         # padded targets per image
NJ = BPC * NT_PAD   # 128 padded targets per core
N_CLS = 80
N_BINS = 16
C = 4 * N_BINS + N_CLS  # 144
S0, S1, S2 = 6400, 1600, 400
WS = (80.0, 40.0, 20.0)
# free-dim offset of each (layer, image) block inside the SBUF feature tile
OFFS = {(0, 0): 0, (0, 1): S0,
        (1, 0): 2 * S0, (1, 1): 2 * S0 + S1,
        (2, 0): 2 * S0 + 2 * S1, (2, 1): 2 * S0 + 2 * S1 + S2}
STOT = 2 * (S0 + S1 + S2)  # 16800
USPLIT = 8
STOT8 = STOT // USPLIT     # 2100

# packed-constant column layout
CP_ID = 0            # [128,128] identity
CP_IOTA = 128        # [128,80] arange
CP_ONES = 208
CP_VALID = 209
CP_VNEG = 210
CP_B = 211           # [128,8] wrapped image index
CP_TWR = 219         # [128,24] wrapped cx, cy, layer
CP_TGT = 243         # [128,6] padded targets, j-ordered
CP_W = 249


def _emit(nc, tc, io, pools, mode="full"):
    pf, pw, pp = pools
    if True:
        # ---- packed constants / targets (one DMA, scalar queue) ----
        cp = pw.tile([128, CP_W], F32, tag="cp")
        nc.scalar.dma_start(cp[:], io["cpack"])
        cid = cp[:, CP_ID:CP_ID + 128]
        ciota = cp[:, CP_IOTA:CP_IOTA + N_CLS]
        cones = cp[:, CP_ONES:CP_ONES + 1]
        cvalid = cp[:, CP_VALID:CP_VALID + 1]
        cvneg = cp[:, CP_VNEG:CP_VNEG + 1]
        cb = cp[:, CP_B:CP_B + 8]
        twr = cp[:, CP_TWR:CP_TWR + 24]
        tg = cp[:, CP_TGT:CP_TGT + 6]

        # ---- stream feature maps into SBUF, both HWDGE queues ----
        # Remainder tile first (small): its gather then hides under the
        # main streaming.  Main channels live in one tile per layer so the
        # per-layer gathers start as soon as their own layer has landed;
        # halves split across the two HWDGE queues (sync + scalar).
        FaL = [pf.tile([128, 2 * S0], F32, name="Fa0", tag="Fa0"),
               pf.tile([128, 2 * S1], F32, name="Fa1", tag="Fa1"),
               pf.tile([128, 2 * S2], F32, name="Fa2", tag="Fa2")]
        Fb = pf.tile([128, STOT8], F32, tag="Fb")  # channels 128..144, u-split
        feats = [io["feat0"], io["feat1"], io["feat2"]]
        for l in range(3):
            for b in range(BPC):
                off = OFFS[(l, b)]
                S = (S0, S1, S2)[l]
                nc.scalar.dma_start(
                    Fb[:, off // USPLIT:(off + S) // USPLIT],
                    feats[l][b, 128:C, :].rearrange("c (u s) -> (c u) s",
                                                    u=USPLIT))
        for l in range(3):
            for b in range(BPC):
                S = (S0, S1, S2)[l]
                h = S // 2
                nc.sync.dma_start(FaL[l][:, b * S:b * S + h],
                                  feats[l][b, 0:128, 0:h])
                nc.scalar.dma_start(FaL[l][:, b * S + h:(b + 1) * S],
                                    feats[l][b, 0:128, h:S])

        # ---- gather-index computation, wrapped layout [128, 8] ----
        cx = twr[:, 0:8]
        cy = twr[:, 8:16]
        ly = twr[:, 16:24]

        def teq(src_ap, val, tag, shape=(128, 8)):
            t = pw.tile(list(shape), F32, tag=tag)
            nc.vector.tensor_scalar(t[:], src_ap, float(val), None, ALU.is_equal)
            return t

        def wsum(es, ws, tag, shape=(128, 8)):
            # sum_i ws[i]*es[i]
            t = pw.tile(list(shape), F32, tag=tag)
            tt = pw.tile(list(shape), F32, tag=tag + "_t")
            nc.vector.tensor_scalar(t[:], es[0][:], ws[0], None, ALU.mult)
            for e, w in zip(es[1:], ws[1:]):
                nc.vector.tensor_scalar(tt[:], e[:], w, None, ALU.mult)
                nc.vector.tensor_add(t[:], t[:], tt[:])
            return t

        e0 = teq(ly, 0.0, "e0")
        e1 = teq(ly, 1.0, "e1")
        e2 = teq(ly, 2.0, "e2")
        es = (e0, e1, e2)
        wt = wsum([e0, e1, e2], [WS[0], WS[1], WS[2]], "wt")
        w8 = wsum([e0, e1, e2], [S0 / 8, S1 / 8, S2 / 8], "w8")
        inv8 = wsum([e0, e1, e2], [8 / S0, 8 / S1, 8 / S2], "inv8")

        # base = e0*(S0*b) + e1*(2*S0 + S1*b) + e2*(2*S0 + 2*S1 + S2*b)
        # (only used for the remainder tile's concatenated layout)
        base = pw.tile([128, 8], F32, tag="base")
        tmp = pw.tile([128, 8], F32, tag="tmp")
        nc.vector.tensor_scalar(tmp[:], cb, float(S0), None, ALU.mult)
        nc.vector.tensor_mul(base[:], tmp[:], e0[:])
        nc.vector.tensor_scalar(tmp[:], cb, float(S1), float(2 * S0),
                                ALU.mult, ALU.add)
        nc.vector.tensor_mul(tmp[:], tmp[:], e1[:])
        nc.vector.tensor_add(base[:], base[:], tmp[:])
        nc.vector.tensor_scalar(tmp[:], cb, float(S2), float(2 * S0 + 2 * S1),
                                ALU.mult, ALU.add)
        nc.vector.tensor_mul(tmp[:], tmp[:], e2[:])
        nc.vector.tensor_add(base[:], base[:], tmp[:])

        def emit_floor(dst, src, itag, shape=(128, 8)):
            # dst = floor(src) for src >= 0, robust to trunc or round casts.
            # Works when dst aliases src (src value is kept in ff).
            ii = pw.tile(list(shape), I32, tag=itag + "_i")
            ff = pw.tile(list(shape), F32, tag=itag + "_f")
            adj = pw.tile(list(shape), F32, tag=itag + "_a")
            nc.vector.tensor_copy(ii[:], src)
            nc.vector.tensor_copy(ff[:], ii[:])
            nc.vector.tensor_tensor(adj[:], ff[:], src, ALU.is_gt)
            nc.vector.tensor_sub(dst, ff[:], adj[:])

        prodx = pw.tile([128, 8], F32, tag="prodx")
        fxv = pw.tile([128, 8], F32, tag="fxv")
        nc.vector.tensor_mul(prodx[:], cx, wt[:])
        emit_floor(fxv[:], prodx[:], "fx")
        prody = pw.tile([128, 8], F32, tag="prody")
        fyv = pw.tile([128, 8], F32, tag="fyv")
        nc.vector.tensor_mul(prody[:], cy, wt[:])
        emit_floor(fyv[:], prody[:], "fy")

        sloc = pw.tile([128, 8], F32, tag="sloc")
        nc.vector.tensor_mul(sloc[:], fyv[:], wt[:])
        nc.vector.tensor_add(sloc[:], sloc[:], fxv[:])
        # per-layer-tile local index: e_l * (b*S_l + s_local)
        sidxL = []
        for l, S in enumerate((S0, S1, S2)):
            sl = pw.tile([128, 8], F32, tag=f"sl{l}")
            nc.vector.tensor_scalar(sl[:], cb, float(S), None, ALU.mult)
            nc.vector.tensor_add(sl[:], sl[:], sloc[:])
            nc.vector.tensor_mul(sl[:], sl[:], es[l][:])
            si = pw.tile([128, 8], I16, tag=f"si{l}")
            nc.vector.tensor_copy(si[:], sl[:])
            sidxL.append(si)

        # u = floor((sloc + 0.5) * inv8); r = base/8 + sloc - u*w8
        uv = pw.tile([128, 8], F32, tag="uv")
        nc.vector.tensor_scalar(uv[:], sloc[:], 0.5, None, ALU.add)
        nc.vector.tensor_mul(uv[:], uv[:], inv8[:])
        emit_floor(uv[:], uv[:], "u")
        rv = pw.tile([128, 8], F32, tag="rv")
        nc.vector.tensor_mul(rv[:], uv[:], w8[:])
        nc.vector.tensor_sub(rv[:], sloc[:], rv[:])
        nc.vector.tensor_scalar(tmp[:], base[:], 1.0 / USPLIT, None, ALU.mult)
        nc.vector.tensor_add(rv[:], rv[:], tmp[:])
        ridx = pw.tile([128, 8], I16, tag="ridx")
        nc.vector.tensor_copy(ridx[:], rv[:])

        if mode == "dma":
            # benchmark variant: streaming only
            osb = pw.tile([2, 1], F32, tag="osb")
            nc.vector.memset(osb[:], 0.0)
            nc.sync.dma_start(io["out"], osb[:])
            return

        # per-target layer masks in loss layout (used by selects below)
        lyp = tg[:, 5:6]
        p0 = teq(lyp, 0.0, "p0", (128, 1))
        p1 = teq(lyp, 1.0, "p1", (128, 1))
        p2 = teq(lyp, 2.0, "p2", (128, 1))
        ps_ = (p0, p1, p2)

        # ---- gather the feature columns of every target ----
        # remainder first (its data lands first), then per-layer main
        G2 = pw.tile([128, NJ], F32, tag="G2")
        nc.gpsimd.ap_gather(G2[:], Fb[:], ridx[:], channels=128,
                            num_elems=STOT8, d=1, num_idxs=NJ)
        TP2 = pp.tile([128, 128], F32, tag="TP2")
        nc.tensor.transpose(TP2[:], G2[:], cid)
        T2r = pw.tile([128, 128], F32, tag="T2r")
        nc.vector.tensor_copy(T2r[:], TP2[:])

        T = pw.tile([128, C], F32, tag="T")
        TPl = []
        for l, S in enumerate((S0, S1, S2)):
            G1l = pw.tile([128, NJ], F32, tag=f"G1{l}")
            nc.gpsimd.ap_gather(G1l[:], FaL[l][:], sidxL[l][:], channels=128,
                                num_elems=2 * S, d=1, num_idxs=NJ)
            tp = pp.tile([128, 128], F32, tag=f"TP{l}")
            nc.tensor.transpose(tp[:], G1l[:], cid)
            TPl.append(tp)
        # T[:, 0:128] = sum_l TP_l * (layer == l), fused on DVE
        selA = pw.tile([128, 128], F32, tag="selA")
        nc.vector.tensor_scalar(selA[:], TPl[0][:], p0[:], None, ALU.mult)
        nc.vector.scalar_tensor_tensor(selA[:], TPl[1][:], p1[:], selA[:],
                                       ALU.mult, ALU.add)
        nc.vector.scalar_tensor_tensor(T[:, 0:128], TPl[2][:], p2[:], selA[:],
                                       ALU.mult, ALU.add)

        if mode == "gather":
            # benchmark variant: streaming + gathers + transposes/selects
            osb = pw.tile([2, 1], F32, tag="osb")
            nc.vector.tensor_copy(osb[:], T[0:2, 0:1])
            nc.sync.dma_start(io["out"], osb[:])
            return

        hh = wsum([p0, p1, p2], [WS[0] / 2, WS[1] / 2, WS[2] / 2], "hh", (128, 1))
        wp = pw.tile([128, 1], F32, tag="wp")
        nc.vector.tensor_scalar(wp[:], hh[:], 2.0, None, ALU.mult)
        invp = wsum([p0, p1, p2], [8 / S0, 8 / S1, 8 / S2], "invp", (128, 1))
        fxp = pw.tile([128, 1], F32, tag="fxp")
        prodp = pw.tile([128, 1], F32, tag="prodp")
        nc.vector.tensor_mul(prodp[:], tg[:, 1:2], wp[:])
        emit_floor(fxp[:], prodp[:], "fxp", (128, 1))
        fyp = pw.tile([128, 1], F32, tag="fyp")
        nc.vector.tensor_mul(prodp[:], tg[:, 2:3], wp[:])
        emit_floor(fyp[:], prodp[:], "fyp", (128, 1))
        sp = pw.tile([128, 1], F32, tag="sp")
        nc.vector.tensor_mul(sp[:], fyp[:], wp[:])
        nc.vector.tensor_add(sp[:], sp[:], fxp[:])
        up = pw.tile([128, 1], F32, tag="up")
        nc.vector.tensor_scalar(up[:], sp[:], 0.5, None, ALU.add)
        nc.vector.tensor_mul(up[:], up[:], invp[:])
        emit_floor(up[:], up[:], "up", (128, 1))

        ohu = pw.tile([128, USPLIT], F32, tag="ohu")
        nc.vector.tensor_tensor(ohu[:], ciota[:, 0:USPLIT],
                                up[:].to_broadcast([128, USPLIT]), ALU.is_equal)
        t2m = pw.tile([128, 128], F32, tag="t2m")
        nc.vector.tensor_tensor(
            t2m[:].rearrange("p (c u) -> p c u", u=USPLIT),
            T2r[:].rearrange("p (c u) -> p c u", u=USPLIT),
            ohu[:].unsqueeze(1).to_broadcast([128, 16, USPLIT]), ALU.mult)
        nc.vector.reduce_sum(T[:, 128:C],
                             t2m[:].rearrange("p (c u) -> p c u", u=USPLIT),
                             axis=AX.X)

        S = pw.tile([128, 2], F32, tag="S")

        # ---- focal classification loss ----
        z = T[:, 64:C]  # [128, 80] logits
        ez = pw.tile([128, N_CLS], F32, tag="ez")
        sez = pw.tile([128, 1], F32, tag="sez")
        i_expz = nc.scalar.activation(ez[:], z, ACT.Exp, accum_out=sez[:])
        # DFL exp right after (same ACT table; avoids a table reload)
        d64 = T[:, 0:64]
        ed = pw.tile([128, 64], F32, tag="ed")
        i_expd = nc.scalar.activation(ed[:], d64, ACT.Exp)
        lse = pw.tile([128, 1], F32, tag="lse")
        i_ln = nc.scalar.activation(lse[:], sez[:], ACT.Ln)
        # keep ACT order Exp,Exp,Ln,Ln so only one table switch happens
        add_dep_helper(i_ln.ins, i_expd.ins, sync=False,
                       reason="group Exp before Ln to avoid table thrash")
        se4 = pw.tile([128, 4], F32, tag="se4")
        nc.vector.reduce_sum(se4[:], ed[:].rearrange("p (a b) -> p a b", b=N_BINS),
                             axis=AX.X)
        lse4 = pw.tile([128, 4], F32, tag="lse4")
        nc.scalar.activation(lse4[:], se4[:], ACT.Ln)

        oh = pw.tile([128, N_CLS], F32, tag="oh")
        nc.vector.tensor_tensor(oh[:], ciota,
                                tg[:, 0:1].to_broadcast([128, N_CLS]),
                                ALU.is_equal)
        zm = pw.tile([128, N_CLS], F32, tag="zm")
        nc.vector.tensor_mul(zm[:], z, oh[:])
        zsel = pw.tile([128, 1], F32, tag="zsel")
        nc.vector.reduce_sum(zsel[:], zm[:], axis=AX.X)
        ce = pw.tile([128, 1], F32, tag="ce")
        nc.vector.tensor_sub(ce[:], lse[:], zsel[:])
        # pt = exp(-ce) = exp(z_sel)/sum(exp(z)) computed on DVE (no 3rd
        # ACT table load): pt = sum(ez*onehot) * recip(sez)
        em = pw.tile([128, N_CLS], F32, tag="em")
        nc.vector.tensor_mul(em[:], ez[:], oh[:])
        esel = pw.tile([128, 1], F32, tag="esel")
        nc.vector.reduce_sum(esel[:], em[:], axis=AX.X)
        rse = pw.tile([128, 1], F32, tag="rse")
        nc.vector.reciprocal(rse[:], sez[:])
        pt = pw.tile([128, 1], F32, tag="pt")
        nc.vector.tensor_mul(pt[:], esel[:], rse[:])
        u1 = pw.tile([128, 1], F32, tag="u1")
        nc.vector.tensor_scalar(u1[:], pt[:], -1.0, 1.0, ALU.mult, ALU.add)
        u2 = pw.tile([128, 1], F32, tag="u2")
        nc.vector.tensor_mul(u2[:], u1[:], u1[:])
        nc.vector.tensor_mul(u2[:], u2[:], ce[:])
        nc.vector.tensor_mul(S[:, 0:1], u2[:], cvalid)

        # ---- DFL box loss ----
        g1 = pw.tile([128, 1], F32, tag="g1")
        g2 = pw.tile([128, 1], F32, tag="g2")
        nc.vector.tensor_mul(g1[:], tg[:, 3:4], hh[:])
        nc.vector.tensor_mul(g2[:], tg[:, 4:5], hh[:])
        t4 = pw.tile([128, 4], F32, tag="t4")
        t4v = t4[:].rearrange("p (a b) -> p a b", b=2)
        nc.vector.tensor_copy(t4v[:, :, 0:1],
                              g1[:].unsqueeze(2).to_broadcast([128, 2, 1]))
        nc.vector.tensor_copy(t4v[:, :, 1:2],
                              g2[:].unsqueeze(2).to_broadcast([128, 2, 1]))
        nc.vector.tensor_scalar(t4[:], t4[:], float(N_BINS - 1 - 1e-06), None,
                                ALU.min)

        li = pw.tile([128, 4], F32, tag="li")
        emit_floor(li[:], t4[:], "li", (128, 4))
        lip = pw.tile([128, 4], F32, tag="lip")
        nc.vector.tensor_scalar(lip[:], li[:], 1.0, None, ALU.add)
        wl = pw.tile([128, 4], F32, tag="wl")
        nc.vector.tensor_sub(wl[:], lip[:], t4[:])
        wr = pw.tile([128, 4], F32, tag="wr")
        nc.vector.tensor_sub(wr[:], t4[:], li[:])

        iota16b = ciota[:, 0:N_BINS].unsqueeze(1).to_broadcast([128, 4, N_BINS])

        def pick(idx, tag):
            ohx = pw.tile([128, 64], F32, tag=tag + "_oh")
            nc.vector.tensor_tensor(
                ohx[:].rearrange("p (a b) -> p a b", b=N_BINS), iota16b,
                idx.unsqueeze(2).to_broadcast([128, 4, N_BINS]), ALU.is_equal)
            dm = pw.tile([128, 64], F32, tag=tag + "_dm")
            nc.vector.tensor_mul(dm[:], d64, ohx[:])
            dsel = pw.tile([128, 4], F32, tag=tag + "_d")
            nc.vector.reduce_sum(dsel[:],
                                 dm[:].rearrange("p (a b) -> p a b", b=N_BINS),
                                 axis=AX.X)
            return dsel

        dl = pick(li[:], "dl")
        dr = pick(lip[:], "dr")
        lpl = pw.tile([128, 4], F32, tag="lpl")
        nc.vector.tensor_sub(lpl[:], dl[:], lse4[:])
        lpr = pw.tile([128, 4], F32, tag="lpr")
        nc.vector.tensor_sub(lpr[:], dr[:], lse4[:])
        nc.vector.tensor_mul(lpl[:], lpl[:], wl[:])
        nc.vector.tensor_mul(lpr[:], lpr[:], wr[:])
        acc = pw.tile([128, 4], F32, tag="acc")
        nc.vector.tensor_add(acc[:], lpl[:], lpr[:])
        boxt = pw.tile([128, 1], F32, tag="boxt")
        nc.vector.reduce_sum(boxt[:], acc[:], axis=AX.X)
        nc.vector.tensor_mul(S[:, 1:2], boxt[:], cvneg)

        # ---- reduce the 128 per-target contributions to 2 scalars ----
        PS = pp.tile([2, 1], F32, tag="PS")
        nc.tensor.matmul(PS[:], S[:], cones, start=True, stop=True)
        osb = pw.tile([2, 1], F32, tag="osb")
        nc.vector.tensor_copy(osb[:], PS[:])
        nc.sync.dma_start(io["out"], osb[:])


_CACHE = {}


def _build(reps=1, mode="full"):
    key = f"nc{reps}_{mode}"
    if key in _CACHE:
        return _CACHE[key], _CACHE[key + "_names"]
    nc = bacc.Bacc("TRN2", target_bir_lowering=False, debug=False,
                   enable_asserts=False, num_devices=N_CORES)
    io = {}

    def din(name, shape, dt=F32):
        io[name] = nc.dram_tensor(name, shape, dt, kind="ExternalInput").ap()

    din("feat0", [BPC, C, S0])
    din("feat1", [BPC, C, S1])
    din("feat2", [BPC, C, S2])
    din("cpack", [128, CP_W])
    io["out"] = nc.dram_tensor("out", [2, 1], F32, kind="ExternalOutput").ap()

    with tile.TileContext(nc) as tc:
        with tc.tile_pool(name="feat", bufs=1) as pf, \
             tc.tile_pool(name="wk", bufs=1) as pw, \
             tc.tile_pool(name="ps", bufs=1, space="PSUM") as pp:
            for r in range(reps):
                if r:
                    # isolate repetitions (timing builds only; reps=1 in prod)
                    tc.strict_bb_all_engine_barrier()
                _emit(nc, tc, io, (pf, pw, pp), mode=mode)
    nc.compile()
    _CACHE[key] = nc
    _CACHE[key + "_names"] = list(io)
    return nc, list(io)


def _const_block():
    if "cblk" in _CACHE:
        return _CACHE["cblk"]
    j = np.arange(NJ)
    blk = np.zeros((128, CP_W - CP_B), np.float32)  # cb..end minus twr/tgt
    cb = ((np.arange(8)[None, :] * 16 + (j[:, None] % 16)) // NT_PAD)
    out = {
        "cid": np.eye(128, dtype=np.float32),
        "ciota": np.broadcast_to(np.arange(N_CLS, dtype=np.float32),
                                 (128, N_CLS)).copy(),
        "cones": np.ones((128, 1), np.float32),
        "cvalid": ((j % NT_PAD) < N_TGT).astype(np.float32)[:, None],
        "cvneg": -((j % NT_PAD) < N_TGT).astype(np.float32)[:, None],
        "cb": cb.astype(np.float32),
    }
    _CACHE["cblk"] = out
    return out


def _per_core_inputs(feat0, feat1, feat2, targets, core):
    b0 = core * BPC
    tpad = np.zeros((BPC, NT_PAD, 6), np.float32)
    tpad[:, :, 5] = 3.0  # pad rows match no layer
    tpad[:, :N_TGT, :] = targets[b0:b0 + BPC]
    tpad = tpad.reshape(NJ, 6)

    # wrapped+replicated layout: w[p, col] = field[col*16 + p%16]
    wi = (np.arange(8)[None, :] * 16 + (np.arange(128)[:, None] % 16))
    twr = np.concatenate([tpad[:, 1][wi], tpad[:, 2][wi], tpad[:, 5][wi]],
                         axis=1).astype(np.float32)

    cb = _const_block()
    cpack = np.empty((128, CP_W), np.float32)
    cpack[:, CP_ID:CP_ID + 128] = cb["cid"]
    cpack[:, CP_IOTA:CP_IOTA + N_CLS] = cb["ciota"]
    cpack[:, CP_ONES:CP_ONES + 1] = cb["cones"]
    cpack[:, CP_VALID:CP_VALID + 1] = cb["cvalid"]
    cpack[:, CP_VNEG:CP_VNEG + 1] = cb["cvneg"]
    cpack[:, CP_B:CP_B + 8] = cb["cb"]
    cpack[:, CP_TWR:CP_TWR + 24] = twr
    cpack[:, CP_TGT:CP_TGT + 6] = tpad

    return {
        "feat0": np.ascontiguousarray(feat0[b0:b0 + BPC].reshape(BPC, C, S0)),
        "feat1": np.ascontiguousarray(feat1[b0:b0 + BPC].reshape(BPC, C, S1)),
        "feat2": np.ascontiguousarray(feat2[b0:b0 + BPC].reshape(BPC, C, S2)),
        "cpack": cpack,
    }


def kernel(feat0, feat1, feat2, targets):
    nc, _ = _build()
    in_maps = [_per_core_inputs(feat0, feat1, feat2, targets, k)
               for k in range(N_CORES)]
    res = run_bass_kernel_spmd(nc, in_maps, core_ids=list(range(N_CORES)))
    parts = np.stack([r["out"].reshape(2) for r in res.results])  # [8, 2]
    cls_sum = np.float32(parts[:, 0].sum(dtype=np.float32))
    box_sum = np.float32(parts[:, 1].sum(dtype=np.float32))
    total = np.float32(cls_sum + box_sum)
    return (total, cls_sum, box_sum)



# revision 14
# speedup vs baseline: 590.8669x; 590.8669x over previous
"""Trainium2 Bass kernel for DetectionLoss (focal cls + DFL box loss).

Strategy
--------
Data-parallel over the batch: 16 images -> 8 cores x 2 images.

The reference loss only reads the feature maps at 50 target locations per
image (each target contributes only at its own FPN layer, because the layer
mask zeroes the other two layers).  Per core we:

  1. Stream the core's full feature-map shard (9.7 MB) into SBUF with large
     contiguous DMAs over both HWDGE queues, SMALL LAYERS FIRST and layer 0
     split into spatial chunks, so the GPSIMD gathers pipeline with the
     streaming instead of serializing after it.
     Channels 0..128 land in per-layer tiles; the remaining 16 channels are
     folded to full partition width as [128, 2100] by splitting each
     (layer, image) spatial block into 8 sub-blocks (partition=(c-128)*8+u).
  2. Compute, on device, per-chunk local gather indices of each (padded)
     target from the raw `targets` tensor (fx = floor(cx*W) etc.), plus
     per-target chunk-selection masks.  All target-only math (one-hots, DFL
     bin weights) is emitted early so it hides under the streaming.
  3. ap_gather (GPSIMD) the feature columns of all 128 padded targets from
     each chunk as soon as its DMA lands; PE-transpose each gathered tile
     and fold it into T[t, c] with per-target chunk/layer masks (DVE).
  4. Focal loss over the 80 class channels + DFL loss over the 4x16 bin
     channels on [128, <=144] tiles (DVE/ACT; Exp table pre-warmed during
     the streams).
  5. Reduce the 128 per-target contributions with a ones-matmul -> [2]
     scalars (cls_sum, box_sum) per core; host sums the 8 partials.

Targets are padded host-side from 50 -> 64 per image with rows whose layer
field is 3 (matches no layer -> masked out; pure padding, no host compute).
"""

import numpy as np

import concourse.bass as bass
import concourse.mybir as mybir
import concourse.tile as tile
from concourse import bacc
from concourse.bass_utils import run_bass_kernel_spmd
from concourse.tile_rust import add_dep_helper

F32 = mybir.dt.float32
I32 = mybir.dt.int32
I16 = mybir.dt.int16
ALU = mybir.AluOpType
ACT = mybir.ActivationFunctionType
AX = mybir.AxisListType

N_CORES = 8
B = 16
BPC = B // N_CORES  # images per core
N_TGT = 50
NT_PAD = 64         # padded targets per image
NJ = BPC * NT_PAD   # 128 padded targets per core
N_CLS = 80
N_BINS = 16
C = 4 * N_BINS + N_CLS  # 144
S0, S1, S2 = 6400, 1600, 400
WS = (80.0, 40.0, 20.0)
STOT = 2 * (S0 + S1 + S2)  # 16800
USPLIT = 8
STOT8 = STOT // USPLIT     # 2100
# free-dim offset of each (layer, image) block inside the remainder tile
OFFS = {(0, 0): 0, (0, 1): S0,
        (1, 0): 2 * S0, (1, 1): 2 * S0 + S1,
        (2, 0): 2 * S0 + 2 * S1, (2, 1): 2 * S0 + 2 * S1 + S2}
NCH = 8                    # layer-0 spatial chunks (of 2*S0)
CHS = 2 * S0 // NCH        # 1600 elements per chunk

# packed-constant column layout
CP_ID = 0            # [128,128] identity
CP_IOTA = 128        # [128,80] arange
CP_ONES = 208
CP_VALID = 209
CP_VNEG = 210
CP_CBT = 211         # per-target image idx (j // NT_PAD)
CP_HW3 = 212         # [W0/2, W1/2, W2/2]
CP_INV3 = 215        # [8/S0, 8/S1, 8/S2]
CP_B = 218           # [128,8] wrapped image index
CP_TWR = 226         # [128,24] wrapped cx, cy, layer
CP_TGT = 250         # [128,6] padded targets, j-ordered
CP_W = 256


def _emit(nc, tc, io, pools, mode="full"):
    pf, pw, pp = pools
    if True:
        # ---- packed constants / targets (first in the DMA queue: the DVE
        # index math and therefore the first gathers depend on it) ----
        cp = pw.tile([128, CP_W], F32, tag="cp")
        nc.sync.dma_start(cp[:], io["cpack"])
        cid = cp[:, CP_ID:CP_ID + 128]
        ciota = cp[:, CP_IOTA:CP_IOTA + N_CLS]
        cones = cp[:, CP_ONES:CP_ONES + 1]
        cvalid = cp[:, CP_VALID:CP_VALID + 1]
        cvneg = cp[:, CP_VNEG:CP_VNEG + 1]
        cbt = cp[:, CP_CBT:CP_CBT + 1]
        chw3 = cp[:, CP_HW3:CP_HW3 + 3]
        cinv3 = cp[:, CP_INV3:CP_INV3 + 3]
        cb = cp[:, CP_B:CP_B + 8]
        twr = cp[:, CP_TWR:CP_TWR + 24]
        tg = cp[:, CP_TGT:CP_TGT + 6]

        # ---- stream feature maps into SBUF ----
        # All on the sync queue so the landing order is deterministic and
        # matches the gather order: remainder channels, then layer 2, layer
        # 1, then layer 0 in NCH fine chunks (so its gather pipelines with
        # its own stream and the final chunk's gather is short).
        FaL = [pf.tile([128, 2 * S0], F32, name="Fa0", tag="Fa0"),
               pf.tile([128, 2 * S1], F32, name="Fa1", tag="Fa1"),
               pf.tile([128, 2 * S2], F32, name="Fa2", tag="Fa2")]
        Fb = pf.tile([128, STOT8], F32, tag="Fb")  # channels 128..144, u-split
        feats = [io["feat0"], io["feat1"], io["feat2"]]
        for b in range(BPC):
            nc.sync.dma_start(FaL[2][:, b * S2:(b + 1) * S2],
                              feats[2][b, 0:128, 0:S2])
        for l in range(3):
            for b in range(BPC):
                off = OFFS[(l, b)]
                S = (S0, S1, S2)[l]
                nc.sync.dma_start(
                    Fb[:, off // USPLIT:(off + S) // USPLIT],
                    feats[l][b, 128:C, :].rearrange("c (u s) -> (c u) s",
                                                    u=USPLIT))
        for b in range(BPC):
            nc.sync.dma_start(FaL[1][:, b * S1:(b + 1) * S1],
                              feats[1][b, 0:128, 0:S1])
        hperb = NCH // BPC           # chunks per image block
        hsz = S0 // hperb            # elements per chunk within one image
        for k in range(NCH):
            b, h = divmod(k, hperb)
            nc.sync.dma_start(
                FaL[0][:, b * S0 + h * hsz:b * S0 + (h + 1) * hsz],
                feats[0][b, 0:128, h * hsz:(h + 1) * hsz])

        # warm the Exp activation table while the streams run (emitted after
        # the scalar-queue dma_starts so its table load doesn't stall them)
        warm = pw.tile([128, 1], F32, tag="warm")
        i_warm = nc.scalar.activation(warm[:], cones, ACT.Exp)

        # ---- gather-index computation, wrapped layout [128, 8] ----
        cx = twr[:, 0:8]
        cy = twr[:, 8:16]
        ly = twr[:, 16:24]

        def teq(src_ap, val, tag, shape=(128, 8)):
            t = pw.tile(list(shape), F32, tag=tag)
            nc.vector.tensor_scalar(t[:], src_ap, float(val), None, ALU.is_equal)
            return t

        def wsum(es, ws, tag, shape=(128, 8)):
            t = pw.tile(list(shape), F32, tag=tag)
            tt = pw.tile(list(shape), F32, tag=tag + "_t")
            nc.vector.tensor_scalar(t[:], es[0][:], ws[0], None, ALU.mult)
            for e, w in zip(es[1:], ws[1:]):
                nc.vector.tensor_scalar(tt[:], e[:], w, None, ALU.mult)
                nc.vector.tensor_add(t[:], t[:], tt[:])
            return t

        e0 = teq(ly, 0.0, "e0")
        e1 = teq(ly, 1.0, "e1")
        e2 = teq(ly, 2.0, "e2")
        es = (e0, e1, e2)
        wt = wsum([e0, e1, e2], [WS[0], WS[1], WS[2]], "wt")

        def emit_floor(dst, src, itag, shape=(128, 8)):
            # dst = floor(src) for src >= 0, robust to trunc or round casts.
            ii = pw.tile(list(shape), I32, tag=itag + "_i")
            ff = pw.tile(list(shape), F32, tag=itag + "_f")
            adj = pw.tile(list(shape), F32, tag=itag + "_a")
            nc.vector.tensor_copy(ii[:], src)
            nc.vector.tensor_copy(ff[:], ii[:])
            nc.vector.tensor_tensor(adj[:], ff[:], src, ALU.is_gt)
            nc.vector.tensor_sub(dst, ff[:], adj[:])

        prodx = pw.tile([128, 8], F32, tag="prodx")
        fxv = pw.tile([128, 8], F32, tag="fxv")
        nc.vector.tensor_mul(prodx[:], cx, wt[:])
        emit_floor(fxv[:], prodx[:], "fx")
        prody = pw.tile([128, 8], F32, tag="prody")
        fyv = pw.tile([128, 8], F32, tag="fyv")
        nc.vector.tensor_mul(prody[:], cy, wt[:])
        emit_floor(fyv[:], prody[:], "fy")

        sloc = pw.tile([128, 8], F32, tag="sloc")
        nc.vector.tensor_mul(sloc[:], fyv[:], wt[:])
        nc.vector.tensor_add(sloc[:], sloc[:], fxv[:])
        # layer-2 local index first (its gather runs first)
        sidxL = {}
        tmp = pw.tile([128, 8], F32, tag="tmp")
        for l, S in ((2, S2), (1, S1)):
            sl = pw.tile([128, 8], F32, tag=f"sl{l}")
            nc.vector.tensor_scalar(sl[:], cb, float(S), None, ALU.mult)
            nc.vector.tensor_add(sl[:], sl[:], sloc[:])
            nc.vector.tensor_mul(sl[:], sl[:], es[l][:])
            si = pw.tile([128, 8], I16, tag=f"si{l}")
            nc.vector.tensor_copy(si[:], sl[:])
            sidxL[l] = si
            if l == 2:
                # remainder-tile index math (G2 gather runs second).
                w8 = wsum([e0, e1, e2], [S0 / 8, S1 / 8, S2 / 8], "w8")
                inv8 = wsum([e0, e1, e2], [8 / S0, 8 / S1, 8 / S2], "inv8")
                # base = e0*(S0*b) + e1*(2*S0+S1*b) + e2*(2*S0+2*S1+S2*b)
                base = pw.tile([128, 8], F32, tag="base")
                nc.vector.tensor_scalar(tmp[:], cb, float(S0), None, ALU.mult)
                nc.vector.tensor_mul(base[:], tmp[:], e0[:])
                nc.vector.tensor_scalar(tmp[:], cb, float(S1), float(2 * S0),
                                        ALU.mult, ALU.add)
                nc.vector.tensor_mul(tmp[:], tmp[:], e1[:])
                nc.vector.tensor_add(base[:], base[:], tmp[:])
                nc.vector.tensor_scalar(tmp[:], cb, float(S2),
                                        float(2 * S0 + 2 * S1),
                                        ALU.mult, ALU.add)
                nc.vector.tensor_mul(tmp[:], tmp[:], e2[:])
                nc.vector.tensor_add(base[:], base[:], tmp[:])
                # u = floor((sloc + 0.5) * inv8); r = base/8 + sloc - u*w8
                uv = pw.tile([128, 8], F32, tag="uv")
                nc.vector.tensor_scalar(uv[:], sloc[:], 0.5, None, ALU.add)
                nc.vector.tensor_mul(uv[:], uv[:], inv8[:])
                emit_floor(uv[:], uv[:], "u")
                rv = pw.tile([128, 8], F32, tag="rv")
                nc.vector.tensor_mul(rv[:], uv[:], w8[:])
                nc.vector.tensor_sub(rv[:], sloc[:], rv[:])
                nc.vector.tensor_scalar(tmp[:], base[:], 1.0 / USPLIT, None,
                                        ALU.mult)
                nc.vector.tensor_add(rv[:], rv[:], tmp[:])
                ridx = pw.tile([128, 8], I16, tag="ridx")
                nc.vector.tensor_copy(ridx[:], rv[:])
        # layer 0: global tile index, chunk id, per-chunk masked local idx
        sl0f = pw.tile([128, 8], F32, tag="sl0f")
        nc.vector.tensor_scalar(sl0f[:], cb, float(S0), None, ALU.mult)
        nc.vector.tensor_add(sl0f[:], sl0f[:], sloc[:])
        kwi = pw.tile([128, 8], I32, tag="kwi")
        kwf = pw.tile([128, 8], F32, tag="kwf")
        nc.vector.tensor_scalar(tmp[:], sl0f[:], 1.0 / CHS, None, ALU.mult)
        nc.vector.tensor_copy(kwi[:], tmp[:])      # trunc (src >= 0)
        nc.vector.tensor_copy(kwf[:], kwi[:])
        sidxC = []
        for k in range(NCH):
            mk = pw.tile([128, 8], F32, tag=f"mk{k}")
            nc.vector.tensor_scalar(mk[:], kwf[:], float(k), None, ALU.is_equal)
            nc.vector.tensor_mul(mk[:], mk[:], e0[:])
            lk = pw.tile([128, 8], F32, tag=f"lk{k}")
            nc.vector.tensor_scalar(lk[:], sl0f[:], float(-k * CHS), None,
                                    ALU.add)
            nc.vector.tensor_mul(lk[:], lk[:], mk[:])
            sik = pw.tile([128, 8], I16, tag=f"sik{k}")
            nc.vector.tensor_copy(sik[:], lk[:])
            sidxC.append(sik)

        if mode == "dma":
            osb = pw.tile([2, 1], F32, tag="osb")
            nc.vector.memset(osb[:], 0.0)
            nc.sync.dma_start(io["out"], osb[:])
            return

        # ---- per-target (loss-layout) masks and DFL/focal pre-computation
        # (depends only on cpack -> runs while the streams are in flight)
        lyp = tg[:, 5:6]
        E3 = pw.tile([128, 3], F32, tag="E3")
        nc.vector.tensor_tensor(E3[:], ciota[:, 0:3],
                                lyp.to_broadcast([128, 3]), ALU.is_equal)
        p0 = E3[:, 0:1]
        p1 = E3[:, 1:2]
        p2 = E3[:, 2:3]
        hw = pw.tile([128, 3], F32, tag="hw")
        nc.vector.tensor_mul(hw[:], E3[:], chw3)
        hh = pw.tile([128, 1], F32, tag="hh")
        nc.vector.reduce_sum(hh[:], hw[:], axis=AX.X)
        wp = pw.tile([128, 1], F32, tag="wp")
        nc.vector.tensor_scalar(wp[:], hh[:], 2.0, None, ALU.mult)
        iv = pw.tile([128, 3], F32, tag="iv")
        nc.vector.tensor_mul(iv[:], E3[:], cinv3)
        invp = pw.tile([128, 1], F32, tag="invp")
        nc.vector.reduce_sum(invp[:], iv[:], axis=AX.X)

        fxp = pw.tile([128, 1], F32, tag="fxp")
        prodp = pw.tile([128, 1], F32, tag="prodp")
        nc.vector.tensor_mul(prodp[:], tg[:, 1:2], wp[:])
        emit_floor(fxp[:], prodp[:], "fxp", (128, 1))
        fyp = pw.tile([128, 1], F32, tag="fyp")
        nc.vector.tensor_mul(prodp[:], tg[:, 2:3], wp[:])
        emit_floor(fyp[:], prodp[:], "fyp", (128, 1))
        sp = pw.tile([128, 1], F32, tag="sp")
        nc.vector.tensor_mul(sp[:], fyp[:], wp[:])
        nc.vector.tensor_add(sp[:], sp[:], fxp[:])
        up = pw.tile([128, 1], F32, tag="up")
        nc.vector.tensor_scalar(up[:], sp[:], 0.5, None, ALU.add)
        nc.vector.tensor_mul(up[:], up[:], invp[:])
        emit_floor(up[:], up[:], "up", (128, 1))
        # layer-0 chunk masks per target: PK[:, k] = (floor(sl0p/CHS)==k)*p0
        sl0p = pw.tile([128, 1], F32, tag="sl0p")
        nc.vector.tensor_scalar(sl0p[:], cbt, float(S0), None, ALU.mult)
        nc.vector.tensor_add(sl0p[:], sl0p[:], sp[:])
        kpi = pw.tile([128, 1], I32, tag="kpi")
        kpf = pw.tile([128, 1], F32, tag="kpf")
        nc.vector.tensor_scalar(sl0p[:], sl0p[:], 1.0 / CHS, None, ALU.mult)
        nc.vector.tensor_copy(kpi[:], sl0p[:])
        nc.vector.tensor_copy(kpf[:], kpi[:])
        PK = pw.tile([128, NCH], F32, tag="PK")
        nc.vector.tensor_tensor(PK[:], ciota[:, 0:NCH],
                                kpf[:].to_broadcast([128, NCH]), ALU.is_equal)
        nc.vector.tensor_scalar(PK[:], PK[:], p0, None, ALU.mult)

        ohu = pw.tile([128, USPLIT], F32, tag="ohu")
        nc.vector.tensor_tensor(ohu[:], ciota[:, 0:USPLIT],
                                up[:].to_broadcast([128, USPLIT]), ALU.is_equal)
        # class one-hot + DFL bin picks (target-only math)
        oh = pw.tile([128, N_CLS], F32, tag="oh")
        nc.vector.tensor_tensor(oh[:], ciota,
                                tg[:, 0:1].to_broadcast([128, N_CLS]),
                                ALU.is_equal)
        g1 = pw.tile([128, 1], F32, tag="g1")
        g2 = pw.tile([128, 1], F32, tag="g2")
        nc.vector.tensor_mul(g1[:], tg[:, 3:4], hh[:])
        nc.vector.tensor_mul(g2[:], tg[:, 4:5], hh[:])
        t4 = pw.tile([128, 4], F32, tag="t4")
        t4v = t4[:].rearrange("p (a b) -> p a b", b=2)
        nc.vector.tensor_copy(t4v[:, :, 0:1],
                              g1[:].unsqueeze(2).to_broadcast([128, 2, 1]))
        nc.vector.tensor_copy(t4v[:, :, 1:2],
                              g2[:].unsqueeze(2).to_broadcast([128, 2, 1]))
        nc.vector.tensor_scalar(t4[:], t4[:], float(N_BINS - 1 - 1e-06), None,
                                ALU.min)
        li = pw.tile([128, 4], F32, tag="li")
        emit_floor(li[:], t4[:], "li", (128, 4))
        lip = pw.tile([128, 4], F32, tag="lip")
        nc.vector.tensor_scalar(lip[:], li[:], 1.0, None, ALU.add)
        wl = pw.tile([128, 4], F32, tag="wl")
        nc.vector.tensor_sub(wl[:], lip[:], t4[:])
        wr = pw.tile([128, 4], F32, tag="wr")
        nc.vector.tensor_sub(wr[:], t4[:], li[:])
        iota16b = ciota[:, 0:N_BINS].unsqueeze(1).to_broadcast([128, 4, N_BINS])
        ohl = pw.tile([128, 64], F32, tag="ohl")
        nc.vector.tensor_tensor(
            ohl[:].rearrange("p (a b) -> p a b", b=N_BINS), iota16b,
            li[:].unsqueeze(2).to_broadcast([128, 4, N_BINS]), ALU.is_equal)
        ohr = pw.tile([128, 64], F32, tag="ohr")
        nc.vector.tensor_tensor(
            ohr[:].rearrange("p (a b) -> p a b", b=N_BINS), iota16b,
            lip[:].unsqueeze(2).to_broadcast([128, 4, N_BINS]), ALU.is_equal)

        # ---- gathers as the streams land; fold into T incrementally ----
        T = pw.tile([128, C], F32, tag="T")
        selA = pw.tile([128, 128], F32, tag="selA")
        # two rotating PSUM tiles: transpose k+1 overlaps the select reading k
        ppt0 = pp.tile([128, 128], F32, name="ppt0", tag="TProt0")
        ppt1 = pp.tile([128, 128], F32, name="ppt1", tag="TProt1")
        ppt = [ppt0, ppt1]
        rot = [0]

        def gather_tr(src_ap, idx, nelem, tag):
            G = pw.tile([128, NJ], F32, tag=f"G{tag}")
            nc.gpsimd.ap_gather(G[:], src_ap, idx[:], channels=128,
                                num_elems=nelem, d=1, num_idxs=NJ)
            tp = ppt[rot[0] % 2]
            rot[0] += 1
            nc.tensor.transpose(tp[:], G[:], cid)
            return tp

        # layer 2 first (its stream lands first)
        tp2 = gather_tr(FaL[2][:], sidxL[2], 2 * S2, "l2")
        nc.vector.tensor_scalar(selA[:], tp2[:], p2, None, ALU.mult)
        # remainder tile
        G2 = pw.tile([128, NJ], F32, tag="G2")
        nc.gpsimd.ap_gather(G2[:], Fb[:], ridx[:], channels=128,
                            num_elems=STOT8, d=1, num_idxs=NJ)
        TP2 = pp.tile([128, 128], F32, tag="TP2")
        nc.tensor.transpose(TP2[:], G2[:], cid)
        T2r = pw.tile([128, 128], F32, tag="T2r")
        nc.vector.tensor_copy(T2r[:], TP2[:])
        t2m = pw.tile([128, 128], F32, tag="t2m")
        nc.vector.tensor_tensor(
            t2m[:].rearrange("p (c u) -> p c u", u=USPLIT),
            T2r[:].rearrange("p (c u) -> p c u", u=USPLIT),
            ohu[:].unsqueeze(1).to_broadcast([128, 16, USPLIT]), ALU.mult)
        nc.vector.reduce_sum(T[:, 128:C],
                             t2m[:].rearrange("p (c u) -> p c u", u=USPLIT),
                             axis=AX.X)
        # layer 1
        tp1 = gather_tr(FaL[1][:], sidxL[1], 2 * S1, "l1")
        nc.vector.scalar_tensor_tensor(selA[:], tp1[:], p1, selA[:],
                                       ALU.mult, ALU.add)
        # layer 0 chunks
        for k in range(NCH):
            tpc = gather_tr(FaL[0][:, k * CHS:(k + 1) * CHS], sidxC[k],
                            CHS, f"c{k}")
            dst = T[:, 0:128] if k == NCH - 1 else selA[:]
            nc.vector.scalar_tensor_tensor(dst, tpc[:], PK[:, k:k + 1],
                                           selA[:], ALU.mult, ALU.add)

        if mode == "gather":
            osb = pw.tile([2, 1], F32, tag="osb")
            nc.vector.tensor_copy(osb[:], T[0:2, 0:1])
            nc.sync.dma_start(io["out"], osb[:])
            return

        S = pw.tile([128, 2], F32, tag="S")

        # ---- focal classification loss ----
        z = T[:, 64:C]  # [128, 80] logits
        ez = pw.tile([128, N_CLS], F32, tag="ez")
        sez = pw.tile([128, 1], F32, tag="sez")
        i_expz = nc.scalar.activation(ez[:], z, ACT.Exp, accum_out=sez[:])
        add_dep_helper(i_expz.ins, i_warm.ins, sync=False,
                       reason="reuse pre-warmed Exp table")
        d64 = T[:, 0:64]
        ed = pw.tile([128, 64], F32, tag="ed")
        i_expd = nc.scalar.activation(ed[:], d64, ACT.Exp)
        # logit select depends only on T -> runs on DVE while ACT computes
        # the exps and reloads the Ln table
        zm = pw.tile([128, N_CLS], F32, tag="zm")
        nc.vector.tensor_mul(zm[:], z, oh[:])
        zsel = pw.tile([128, 1], F32, tag="zsel")
        nc.vector.reduce_sum(zsel[:], zm[:], axis=AX.X)
        lse = pw.tile([128, 1], F32, tag="lse")
        i_ln = nc.scalar.activation(lse[:], sez[:], ACT.Ln)
        # keep ACT order Exp,Exp,Ln,Ln so only one table switch happens
        add_dep_helper(i_ln.ins, i_expd.ins, sync=False,
                       reason="group Exp before Ln to avoid table thrash")
        se4 = pw.tile([128, 4], F32, tag="se4")
        nc.vector.reduce_sum(se4[:], ed[:].rearrange("p (a b) -> p a b", b=N_BINS),
                             axis=AX.X)
        lse4 = pw.tile([128, 4], F32, tag="lse4")
        nc.scalar.activation(lse4[:], se4[:], ACT.Ln)

        ce = pw.tile([128, 1], F32, tag="ce")
        nc.vector.tensor_sub(ce[:], lse[:], zsel[:])
        # pt = exp(-ce) computed on DVE: pt = sum(ez*onehot) * recip(sez)
        em = pw.tile([128, N_CLS], F32, tag="em")
        nc.vector.tensor_mul(em[:], ez[:], oh[:])
        esel = pw.tile([128, 1], F32, tag="esel")
        nc.vector.reduce_sum(esel[:], em[:], axis=AX.X)
        rse = pw.tile([128, 1], F32, tag="rse")
        nc.vector.reciprocal(rse[:], sez[:])
        pt = pw.tile([128, 1], F32, tag="pt")
        nc.vector.tensor_mul(pt[:], esel[:], rse[:])
        u1 = pw.tile([128, 1], F32, tag="u1")
        nc.vector.tensor_scalar(u1[:], pt[:], -1.0, 1.0, ALU.mult, ALU.add)
        u2 = pw.tile([128, 1], F32, tag="u2")
        nc.vector.tensor_mul(u2[:], u1[:], u1[:])
        nc.vector.tensor_mul(u2[:], u2[:], ce[:])
        nc.vector.tensor_mul(S[:, 0:1], u2[:], cvalid)

        # ---- DFL box loss ----
        def pick(ohx, tag):
            dm = pw.tile([128, 64], F32, tag=tag + "_dm")
            nc.vector.tensor_mul(dm[:], d64, ohx[:])
            dsel = pw.tile([128, 4], F32, tag=tag + "_d")
            nc.vector.reduce_sum(dsel[:],
                                 dm[:].rearrange("p (a b) -> p a b", b=N_BINS),
                                 axis=AX.X)
            return dsel

        dl = pick(ohl, "dl")
        dr = pick(ohr, "dr")
        # (dl-lse4)*wl + (dr-lse4)*wr == dl*wl + dr*wr - lse4  (wl+wr == 1)
        lpl = pw.tile([128, 4], F32, tag="lpl")
        nc.vector.tensor_mul(lpl[:], dl[:], wl[:])
        lpr = pw.tile([128, 4], F32, tag="lpr")
        nc.vector.tensor_mul(lpr[:], dr[:], wr[:])
        acc = pw.tile([128, 4], F32, tag="acc")
        nc.vector.tensor_add(acc[:], lpl[:], lpr[:])
        nc.vector.tensor_sub(acc[:], acc[:], lse4[:])
        boxt = pw.tile([128, 1], F32, tag="boxt")
        nc.vector.reduce_sum(boxt[:], acc[:], axis=AX.X)
        nc.vector.tensor_mul(S[:, 1:2], boxt[:], cvneg)

        # ---- reduce the 128 per-target contributions to 2 scalars ----
        PS = pp.tile([2, 1], F32, tag="PS")
        nc.tensor.matmul(PS[:], S[:], cones, start=True, stop=True)
        osb = pw.tile([2, 1], F32, tag="osb")
        nc.vector.tensor_copy(osb[:], PS[:])
        nc.sync.dma_start(io["out"], osb[:])


_CACHE = {}


def _build(reps=1, mode="full"):
    key = f"nc{reps}_{mode}"
    if key in _CACHE:
        return _CACHE[key], _CACHE[key + "_names"]
    nc = bacc.Bacc("TRN2", target_bir_lowering=False, debug=False,
                   enable_asserts=False, num_devices=N_CORES)
    io = {}

    def din(name, shape, dt=F32):
        io[name] = nc.dram_tensor(name, shape, dt, kind="ExternalInput").ap()

    din("feat0", [BPC, C, S0])
    din("feat1", [BPC, C, S1])
    din("feat2", [BPC, C, S2])
    din("cpack", [128, CP_W])
    io["out"] = nc.dram_tensor("out", [2, 1], F32, kind="ExternalOutput").ap()

    with tile.TileContext(nc) as tc:
        with tc.tile_pool(name="feat", bufs=1) as pf, \
             tc.tile_pool(name="wk", bufs=1) as pw, \
             tc.tile_pool(name="ps", bufs=1, space="PSUM") as pp:
            for r in range(reps):
                if r:
                    # isolate repetitions (timing builds only; reps=1 in prod)
                    tc.strict_bb_all_engine_barrier()
                _emit(nc, tc, io, (pf, pw, pp), mode=mode)
    nc.compile()
    _CACHE[key] = nc
    _CACHE[key + "_names"] = list(io)
    return nc, list(io)


def _const_block():
    if "cblk" in _CACHE:
        return _CACHE["cblk"]
    j = np.arange(NJ)
    cb = ((np.arange(8)[None, :] * 16 + (j[:, None] % 16)) // NT_PAD)
    out = {
        "cid": np.eye(128, dtype=np.float32),
        "ciota": np.broadcast_to(np.arange(N_CLS, dtype=np.float32),
                                 (128, N_CLS)).copy(),
        "cones": np.ones((128, 1), np.float32),
        "cvalid": ((j % NT_PAD) < N_TGT).astype(np.float32)[:, None],
        "cvneg": -((j % NT_PAD) < N_TGT).astype(np.float32)[:, None],
        "cbt": (j // NT_PAD).astype(np.float32)[:, None],
        "chw3": np.broadcast_to(
            np.array([WS[0] / 2, WS[1] / 2, WS[2] / 2], np.float32),
            (128, 3)).copy(),
        "cinv3": np.broadcast_to(
            np.array([8 / S0, 8 / S1, 8 / S2], np.float32), (128, 3)).copy(),
        "cb": cb.astype(np.float32),
    }
    _CACHE["cblk"] = out
    return out


def _per_core_inputs(feat0, feat1, feat2, targets, core):
    b0 = core * BPC
    tpad = np.zeros((BPC, NT_PAD, 6), np.float32)
    tpad[:, :, 5] = 3.0  # pad rows match no layer
    tpad[:, :N_TGT, :] = targets[b0:b0 + BPC]
    tpad = tpad.reshape(NJ, 6)

    # wrapped+replicated layout: w[p, col] = field[col*16 + p%16]
    wi = (np.arange(8)[None, :] * 16 + (np.arange(128)[:, None] % 16))
    twr = np.concatenate([tpad[:, 1][wi], tpad[:, 2][wi], tpad[:, 5][wi]],
                         axis=1).astype(np.float32)

    cb = _const_block()
    cpack = np.empty((128, CP_W), np.float32)
    cpack[:, CP_ID:CP_ID + 128] = cb["cid"]
    cpack[:, CP_IOTA:CP_IOTA + N_CLS] = cb["ciota"]
    cpack[:, CP_ONES:CP_ONES + 1] = cb["cones"]
    cpack[:, CP_VALID:CP_VALID + 1] = cb["cvalid"]
    cpack[:, CP_VNEG:CP_VNEG + 1] = cb["cvneg"]
    cpack[:, CP_CBT:CP_CBT + 1] = cb["cbt"]
    cpack[:, CP_HW3:CP_HW3 + 3] = cb["chw3"]
    cpack[:, CP_INV3:CP_INV3 + 3] = cb["cinv3"]
    cpack[:, CP_B:CP_B + 8] = cb["cb"]
    cpack[:, CP_TWR:CP_TWR + 24] = twr
    cpack[:, CP_TGT:CP_TGT + 6] = tpad

    return {
        "feat0": np.ascontiguousarray(feat0[b0:b0 + BPC].reshape(BPC, C, S0)),
        "feat1": np.ascontiguousarray(feat1[b0:b0 + BPC].reshape(BPC, C, S1)),
        "feat2": np.ascontiguousarray(feat2[b0:b0 + BPC].reshape(BPC, C, S2)),
        "cpack": cpack,
    }


def kernel(feat0, feat1, feat2, targets):
    nc, _ = _build()
    in_maps = [_per_core_inputs(feat0, feat1, feat2, targets, k)
               for k in range(N_CORES)]
    res = run_bass_kernel_spmd(nc, in_maps, core_ids=list(range(N_CORES)))
    parts = np.stack([r["out"].reshape(2) for r in res.results])  # [8, 2]
    cls_sum = np.float32(parts[:, 0].sum(dtype=np.float32))
    box_sum = np.float32(parts[:, 1].sum(dtype=np.float32))
    total = np.float32(cls_sum + box_sum)
    return (total, cls_sum, box_sum)


# revision 24
# speedup vs baseline: 624.0184x; 1.0561x over previous
"""Trainium2 Bass kernel for DetectionLoss (focal cls + DFL box loss).

Strategy
--------
Data-parallel over the batch: 16 images -> 8 cores x 2 images.

The reference loss only reads the feature maps at 50 target locations per
image (each target contributes only at its own FPN layer, because the layer
mask zeroes the other two layers).  Per core we:

  1. Stream the core's full feature-map shard (9.7 MB) into SBUF with large
     contiguous DMAs over both HWDGE queues, SMALL LAYERS FIRST and layer 0
     split into spatial chunks, so the GPSIMD gathers pipeline with the
     streaming instead of serializing after it.
     Channels 0..128 land in per-layer tiles; the remaining 16 channels are
     folded to full partition width as [128, 2100] by splitting each
     (layer, image) spatial block into 8 sub-blocks (partition=(c-128)*8+u).
  2. Compute, on device, per-chunk local gather indices of each (padded)
     target from the raw `targets` tensor (fx = floor(cx*W) etc.), plus
     per-target chunk-selection masks.  All target-only math (one-hots, DFL
     bin weights) is emitted early so it hides under the streaming.
  3. ap_gather (GPSIMD) the feature columns of all 128 padded targets from
     each chunk as soon as its DMA lands; PE-transpose each gathered tile
     and fold it into T[t, c] with per-target chunk/layer masks (DVE).
  4. Focal loss over the 80 class channels + DFL loss over the 4x16 bin
     channels on [128, <=144] tiles (DVE/ACT; Exp table pre-warmed during
     the streams).
  5. Reduce the 128 per-target contributions with a ones-matmul -> [2]
     scalars (cls_sum, box_sum) per core; host sums the 8 partials.

Targets are padded host-side from 50 -> 64 per image with rows whose layer
field is 3 (matches no layer -> masked out; pure padding, no host compute).
"""

import numpy as np

import concourse.bass as bass
import concourse.mybir as mybir
import concourse.tile as tile
from concourse import bacc
from concourse.bass_utils import run_bass_kernel_spmd
from concourse.tile_rust import add_dep_helper

F32 = mybir.dt.float32
I32 = mybir.dt.int32
I16 = mybir.dt.int16
ALU = mybir.AluOpType
ACT = mybir.ActivationFunctionType
AX = mybir.AxisListType

N_CORES = 8
B = 16
BPC = B // N_CORES  # images per core
N_TGT = 50
NT_PAD = 64         # padded targets per image
NJ = BPC * NT_PAD   # 128 padded targets per core
N_CLS = 80
N_BINS = 16
C = 4 * N_BINS + N_CLS  # 144
S0, S1, S2 = 6400, 1600, 400
WS = (80.0, 40.0, 20.0)
STOT = 2 * (S0 + S1 + S2)  # 16800
USPLIT = 8
STOT8 = STOT // USPLIT     # 2100
# free-dim offset of each (layer, image) block inside the remainder tile
OFFS = {(0, 0): 0, (0, 1): S0,
        (1, 0): 2 * S0, (1, 1): 2 * S0 + S1,
        (2, 0): 2 * S0 + 2 * S1, (2, 1): 2 * S0 + 2 * S1 + S2}
NCH = 8                    # layer-0 spatial chunks (of 2*S0)
CHS = 2 * S0 // NCH        # 1600 elements per chunk

# packed-constant column layout
CP_ID = 0            # [128,128] identity
CP_IOTA = 128        # [128,80] arange
CP_ONES = 208
CP_VALID = 209
CP_VNEG = 210
CP_CBT = 211         # per-target image idx (j // NT_PAD)
CP_HW3 = 212         # [W0/2, W1/2, W2/2]
CP_INV3 = 215        # [8/S0, 8/S1, 8/S2]
CP_B = 218           # [128,8] wrapped image index
CP_TWR = 226         # [128,24] wrapped cx, cy, layer
CP_TGT = 250         # [128,6] padded targets, j-ordered
CP_W = 256


def _emit(nc, tc, io, pools, mode="full"):
    pf, pw, pp = pools
    if True:
        # ---- packed constants / targets (first in the DMA queue: the DVE
        # index math and therefore the first gathers depend on it) ----
        cp = pw.tile([128, CP_W], F32, tag="cp")
        nc.sync.dma_start(cp[:], io["cpack"])
        cid = cp[:, CP_ID:CP_ID + 128]
        ciota = cp[:, CP_IOTA:CP_IOTA + N_CLS]
        cones = cp[:, CP_ONES:CP_ONES + 1]
        cvalid = cp[:, CP_VALID:CP_VALID + 1]
        cvneg = cp[:, CP_VNEG:CP_VNEG + 1]
        cbt = cp[:, CP_CBT:CP_CBT + 1]
        chw3 = cp[:, CP_HW3:CP_HW3 + 3]
        cinv3 = cp[:, CP_INV3:CP_INV3 + 3]
        cb = cp[:, CP_B:CP_B + 8]
        twr = cp[:, CP_TWR:CP_TWR + 24]
        tg = cp[:, CP_TGT:CP_TGT + 6]

        # ---- stream feature maps into SBUF ----
        # All on the sync queue so the landing order is deterministic and
        # matches the gather order: remainder channels, then layer 2, layer
        # 1, then layer 0 in NCH fine chunks (so its gather pipelines with
        # its own stream and the final chunk's gather is short).
        FaL = [pf.tile([128, 2 * S0], F32, name="Fa0", tag="Fa0"),
               pf.tile([128, 2 * S1], F32, name="Fa1", tag="Fa1"),
               pf.tile([128, 2 * S2], F32, name="Fa2", tag="Fa2")]
        Fb = pf.tile([128, STOT8], F32, tag="Fb")  # channels 128..144, u-split
        feats = [io["feat0"], io["feat1"], io["feat2"]]
        for b in range(BPC):
            nc.sync.dma_start(FaL[2][:, b * S2:(b + 1) * S2],
                              feats[2][b, 0:128, 0:S2])
        for b in range(BPC):
            nc.sync.dma_start(FaL[1][:, b * S1:(b + 1) * S1],
                              feats[1][b, 0:128, 0:S1])
        for l in range(3):
            for b in range(BPC):
                off = OFFS[(l, b)]
                S = (S0, S1, S2)[l]
                nc.sync.dma_start(
                    Fb[:, off // USPLIT:(off + S) // USPLIT],
                    feats[l][b, 128:C, :].rearrange("c (u s) -> (c u) s",
                                                    u=USPLIT))
        hperb = NCH // BPC           # chunks per image block
        hsz = S0 // hperb            # elements per chunk within one image
        for k in range(NCH):
            b, h = divmod(k, hperb)
            nc.sync.dma_start(
                FaL[0][:, b * S0 + h * hsz:b * S0 + (h + 1) * hsz],
                feats[0][b, 0:128, h * hsz:(h + 1) * hsz])

        # warm the Exp activation table while the streams run (emitted after
        # the scalar-queue dma_starts so its table load doesn't stall them)
        warm = pw.tile([128, 1], F32, tag="warm")
        i_warm = nc.scalar.activation(warm[:], cones, ACT.Exp)

        # ---- gather-index computation, wrapped layout [128, 8] ----
        cx = twr[:, 0:8]
        cy = twr[:, 8:16]
        ly = twr[:, 16:24]

        def teq(src_ap, val, tag, shape=(128, 8)):
            t = pw.tile(list(shape), F32, tag=tag)
            nc.vector.tensor_scalar(t[:], src_ap, float(val), None, ALU.is_equal)
            return t

        def wsum(es, ws, tag, shape=(128, 8)):
            t = pw.tile(list(shape), F32, tag=tag)
            tt = pw.tile(list(shape), F32, tag=tag + "_t")
            nc.vector.tensor_scalar(t[:], es[0][:], ws[0], None, ALU.mult)
            for e, w in zip(es[1:], ws[1:]):
                nc.vector.tensor_scalar(tt[:], e[:], w, None, ALU.mult)
                nc.vector.tensor_add(t[:], t[:], tt[:])
            return t

        e0 = teq(ly, 0.0, "e0")
        e1 = teq(ly, 1.0, "e1")
        e2 = teq(ly, 2.0, "e2")
        es = (e0, e1, e2)
        wt = wsum([e0, e1, e2], [WS[0], WS[1], WS[2]], "wt")

        def emit_floor(dst, src, itag, shape=(128, 8)):
            # dst = floor(src) for src >= 0, robust to trunc or round casts.
            ii = pw.tile(list(shape), I32, tag=itag + "_i")
            ff = pw.tile(list(shape), F32, tag=itag + "_f")
            adj = pw.tile(list(shape), F32, tag=itag + "_a")
            nc.vector.tensor_copy(ii[:], src)
            nc.vector.tensor_copy(ff[:], ii[:])
            nc.vector.tensor_tensor(adj[:], ff[:], src, ALU.is_gt)
            nc.vector.tensor_sub(dst, ff[:], adj[:])

        prodx = pw.tile([128, 8], F32, tag="prodx")
        fxv = pw.tile([128, 8], F32, tag="fxv")
        nc.vector.tensor_mul(prodx[:], cx, wt[:])
        emit_floor(fxv[:], prodx[:], "fx")
        prody = pw.tile([128, 8], F32, tag="prody")
        fyv = pw.tile([128, 8], F32, tag="fyv")
        nc.vector.tensor_mul(prody[:], cy, wt[:])
        emit_floor(fyv[:], prody[:], "fy")

        sloc = pw.tile([128, 8], F32, tag="sloc")
        nc.vector.tensor_mul(sloc[:], fyv[:], wt[:])
        nc.vector.tensor_add(sloc[:], sloc[:], fxv[:])
        tmp = pw.tile([128, 8], F32, tag="tmp")
        # layer-2 local index first (its gather runs first)
        sidxL = {}
        sl2 = pw.tile([128, 8], F32, tag="sl2")
        nc.vector.tensor_scalar(sl2[:], cb, float(S2), None, ALU.mult)
        nc.vector.tensor_add(sl2[:], sl2[:], sloc[:])
        nc.vector.tensor_mul(sl2[:], sl2[:], e2[:])
        si2 = pw.tile([128, 8], I16, tag="si2")
        nc.vector.tensor_copy(si2[:], sl2[:])
        sidxL[2] = si2
        # layer-1 per-image local indices (one gather per image block, so
        # each starts as soon as its own block has landed)
        sidx1b = []
        for b in range(BPC):
            ebb = pw.tile([128, 8], F32, tag=f"ebb{b}")
            nc.vector.tensor_scalar(ebb[:], cb, float(b), None, ALU.is_equal)
            nc.vector.tensor_mul(ebb[:], ebb[:], e1[:])
            slb = pw.tile([128, 8], F32, tag=f"sl1b{b}")
            nc.vector.tensor_mul(slb[:], sloc[:], ebb[:])
            sib = pw.tile([128, 8], I16, tag=f"si1b{b}")
            nc.vector.tensor_copy(sib[:], slb[:])
            sidx1b.append(sib)
        # remainder-tile index math (G2 gather runs after layer 1).
        w8 = wsum([e0, e1, e2], [S0 / 8, S1 / 8, S2 / 8], "w8")
        inv8 = wsum([e0, e1, e2], [8 / S0, 8 / S1, 8 / S2], "inv8")
        # base = e0*(S0*b) + e1*(2*S0+S1*b) + e2*(2*S0+2*S1+S2*b)
        base = pw.tile([128, 8], F32, tag="base")
        nc.vector.tensor_scalar(tmp[:], cb, float(S0), None, ALU.mult)
        nc.vector.tensor_mul(base[:], tmp[:], e0[:])
        nc.vector.tensor_scalar(tmp[:], cb, float(S1), float(2 * S0),
                                ALU.mult, ALU.add)
        nc.vector.tensor_mul(tmp[:], tmp[:], e1[:])
        nc.vector.tensor_add(base[:], base[:], tmp[:])
        nc.vector.tensor_scalar(tmp[:], cb, float(S2),
                                float(2 * S0 + 2 * S1),
                                ALU.mult, ALU.add)
        nc.vector.tensor_mul(tmp[:], tmp[:], e2[:])
        nc.vector.tensor_add(base[:], base[:], tmp[:])
        # u = floor((sloc + 0.5) * inv8); r = base/8 + sloc - u*w8
        uv = pw.tile([128, 8], F32, tag="uv")
        nc.vector.tensor_scalar(uv[:], sloc[:], 0.5, None, ALU.add)
        nc.vector.tensor_mul(uv[:], uv[:], inv8[:])
        emit_floor(uv[:], uv[:], "u")
        rv = pw.tile([128, 8], F32, tag="rv")
        nc.vector.tensor_mul(rv[:], uv[:], w8[:])
        nc.vector.tensor_sub(rv[:], sloc[:], rv[:])
        nc.vector.tensor_scalar(tmp[:], base[:], 1.0 / USPLIT, None,
                                ALU.mult)
        nc.vector.tensor_add(rv[:], rv[:], tmp[:])
        ridx = pw.tile([128, 8], I16, tag="ridx")
        nc.vector.tensor_copy(ridx[:], rv[:])
        # layer 0: global tile index, chunk id, per-chunk masked local idx
        sl0f = pw.tile([128, 8], F32, tag="sl0f")
        nc.vector.tensor_scalar(sl0f[:], cb, float(S0), None, ALU.mult)
        nc.vector.tensor_add(sl0f[:], sl0f[:], sloc[:])
        kwi = pw.tile([128, 8], I32, tag="kwi")
        kwf = pw.tile([128, 8], F32, tag="kwf")
        nc.vector.tensor_scalar(tmp[:], sl0f[:], 1.0 / CHS, None, ALU.mult)
        nc.vector.tensor_copy(kwi[:], tmp[:])      # trunc (src >= 0)
        nc.vector.tensor_copy(kwf[:], kwi[:])
        sidxC = []
        for k in range(NCH):
            mk = pw.tile([128, 8], F32, tag=f"mk{k}")
            nc.vector.tensor_scalar(mk[:], kwf[:], float(k), None, ALU.is_equal)
            nc.vector.tensor_mul(mk[:], mk[:], e0[:])
            lk = pw.tile([128, 8], F32, tag=f"lk{k}")
            nc.vector.tensor_scalar(lk[:], sl0f[:], float(-k * CHS), None,
                                    ALU.add)
            nc.vector.tensor_mul(lk[:], lk[:], mk[:])
            sik = pw.tile([128, 8], I16, tag=f"sik{k}")
            nc.vector.tensor_copy(sik[:], lk[:])
            sidxC.append(sik)

        if mode == "dma":
            osb = pw.tile([2, 1], F32, tag="osb")
            nc.vector.memset(osb[:], 0.0)
            nc.sync.dma_start(io["out"], osb[:])
            return

        # ---- per-target (loss-layout) masks and DFL/focal pre-computation
        # (depends only on cpack -> runs while the streams are in flight)
        lyp = tg[:, 5:6]
        E3 = pw.tile([128, 3], F32, tag="E3")
        nc.vector.tensor_tensor(E3[:], ciota[:, 0:3],
                                lyp.to_broadcast([128, 3]), ALU.is_equal)
        p0 = E3[:, 0:1]
        p1 = E3[:, 1:2]
        p2 = E3[:, 2:3]
        # per-target (layer==1 && image==b) masks for the split L1 gathers
        PB1 = pw.tile([128, BPC], F32, tag="PB1")
        nc.vector.tensor_tensor(PB1[:], ciota[:, 0:BPC],
                                cbt.to_broadcast([128, BPC]), ALU.is_equal)
        nc.vector.tensor_scalar(PB1[:], PB1[:], p1, None, ALU.mult)
        hw = pw.tile([128, 3], F32, tag="hw")
        nc.vector.tensor_mul(hw[:], E3[:], chw3)
        hh = pw.tile([128, 1], F32, tag="hh")
        nc.vector.reduce_sum(hh[:], hw[:], axis=AX.X)
        wp = pw.tile([128, 1], F32, tag="wp")
        nc.vector.tensor_scalar(wp[:], hh[:], 2.0, None, ALU.mult)
        iv = pw.tile([128, 3], F32, tag="iv")
        nc.vector.tensor_mul(iv[:], E3[:], cinv3)
        invp = pw.tile([128, 1], F32, tag="invp")
        nc.vector.reduce_sum(invp[:], iv[:], axis=AX.X)

        fxp = pw.tile([128, 1], F32, tag="fxp")
        prodp = pw.tile([128, 1], F32, tag="prodp")
        nc.vector.tensor_mul(prodp[:], tg[:, 1:2], wp[:])
        emit_floor(fxp[:], prodp[:], "fxp", (128, 1))
        fyp = pw.tile([128, 1], F32, tag="fyp")
        nc.vector.tensor_mul(prodp[:], tg[:, 2:3], wp[:])
        emit_floor(fyp[:], prodp[:], "fyp", (128, 1))
        sp = pw.tile([128, 1], F32, tag="sp")
        nc.vector.tensor_mul(sp[:], fyp[:], wp[:])
        nc.vector.tensor_add(sp[:], sp[:], fxp[:])
        up = pw.tile([128, 1], F32, tag="up")
        nc.vector.tensor_scalar(up[:], sp[:], 0.5, None, ALU.add)
        nc.vector.tensor_mul(up[:], up[:], invp[:])
        emit_floor(up[:], up[:], "up", (128, 1))
        # layer-0 chunk masks per target: PK[:, k] = (floor(sl0p/CHS)==k)*p0
        sl0p = pw.tile([128, 1], F32, tag="sl0p")
        nc.vector.tensor_scalar(sl0p[:], cbt, float(S0), None, ALU.mult)
        nc.vector.tensor_add(sl0p[:], sl0p[:], sp[:])
        kpi = pw.tile([128, 1], I32, tag="kpi")
        kpf = pw.tile([128, 1], F32, tag="kpf")
        nc.vector.tensor_scalar(sl0p[:], sl0p[:], 1.0 / CHS, None, ALU.mult)
        nc.vector.tensor_copy(kpi[:], sl0p[:])
        nc.vector.tensor_copy(kpf[:], kpi[:])
        PK = pw.tile([128, NCH], F32, tag="PK")
        nc.vector.tensor_tensor(PK[:], ciota[:, 0:NCH],
                                kpf[:].to_broadcast([128, NCH]), ALU.is_equal)
        nc.vector.tensor_scalar(PK[:], PK[:], p0, None, ALU.mult)

        ohu = pw.tile([128, USPLIT], F32, tag="ohu")
        nc.vector.tensor_tensor(ohu[:], ciota[:, 0:USPLIT],
                                up[:].to_broadcast([128, USPLIT]), ALU.is_equal)
        # class one-hot + DFL bin picks (target-only math)
        oh = pw.tile([128, N_CLS], F32, tag="oh")
        nc.vector.tensor_tensor(oh[:], ciota,
                                tg[:, 0:1].to_broadcast([128, N_CLS]),
                                ALU.is_equal)
        g1 = pw.tile([128, 1], F32, tag="g1")
        g2 = pw.tile([128, 1], F32, tag="g2")
        nc.vector.tensor_mul(g1[:], tg[:, 3:4], hh[:])
        nc.vector.tensor_mul(g2[:], tg[:, 4:5], hh[:])
        t4 = pw.tile([128, 4], F32, tag="t4")
        t4v = t4[:].rearrange("p (a b) -> p a b", b=2)
        nc.vector.tensor_copy(t4v[:, :, 0:1],
                              g1[:].unsqueeze(2).to_broadcast([128, 2, 1]))
        nc.vector.tensor_copy(t4v[:, :, 1:2],
                              g2[:].unsqueeze(2).to_broadcast([128, 2, 1]))
        nc.vector.tensor_scalar(t4[:], t4[:], float(N_BINS - 1 - 1e-06), None,
                                ALU.min)
        li = pw.tile([128, 4], F32, tag="li")
        emit_floor(li[:], t4[:], "li", (128, 4))
        lip = pw.tile([128, 4], F32, tag="lip")
        nc.vector.tensor_scalar(lip[:], li[:], 1.0, None, ALU.add)
        wl = pw.tile([128, 4], F32, tag="wl")
        nc.vector.tensor_sub(wl[:], lip[:], t4[:])
        wr = pw.tile([128, 4], F32, tag="wr")
        nc.vector.tensor_sub(wr[:], t4[:], li[:])
        iota16b = ciota[:, 0:N_BINS].unsqueeze(1).to_broadcast([128, 4, N_BINS])
        ohl = pw.tile([128, 64], F32, tag="ohl")
        nc.vector.tensor_tensor(
            ohl[:].rearrange("p (a b) -> p a b", b=N_BINS), iota16b,
            li[:].unsqueeze(2).to_broadcast([128, 4, N_BINS]), ALU.is_equal)
        ohr = pw.tile([128, 64], F32, tag="ohr")
        nc.vector.tensor_tensor(
            ohr[:].rearrange("p (a b) -> p a b", b=N_BINS), iota16b,
            lip[:].unsqueeze(2).to_broadcast([128, 4, N_BINS]), ALU.is_equal)

        # ---- gathers as the streams land; fold into T incrementally ----
        T = pw.tile([128, C], F32, tag="T")
        selA = pw.tile([128, 128], F32, tag="selA")
        # two rotating PSUM tiles: transpose k+1 overlaps the select reading k
        ppt0 = pp.tile([128, 128], F32, name="ppt0", tag="TProt0")
        ppt1 = pp.tile([128, 128], F32, name="ppt1", tag="TProt1")
        ppt = [ppt0, ppt1]
        rot = [0]

        def gather_tr(src_ap, idx, nelem, tag):
            G = pw.tile([128, NJ], F32, tag=f"G{tag}")
            nc.gpsimd.ap_gather(G[:], src_ap, idx[:], channels=128,
                                num_elems=nelem, d=1, num_idxs=NJ)
            tp = ppt[rot[0] % 2]
            rot[0] += 1
            nc.tensor.transpose(tp[:], G[:], cid)
            return tp

        # layer 2 first (its stream lands first)
        tp2 = gather_tr(FaL[2][:], sidxL[2], 2 * S2, "l2")
        nc.vector.tensor_scalar(selA[:], tp2[:], p2, None, ALU.mult)
        # layer 1, one gather per image block (starts as its block lands)
        for b in range(BPC):
            tp1 = gather_tr(FaL[1][:, b * S1:(b + 1) * S1], sidx1b[b],
                            S1, f"l1{b}")
            nc.vector.scalar_tensor_tensor(selA[:], tp1[:], PB1[:, b:b + 1],
                                           selA[:], ALU.mult, ALU.add)
        # remainder tile
        G2 = pw.tile([128, NJ], F32, tag="G2")
        nc.gpsimd.ap_gather(G2[:], Fb[:], ridx[:], channels=128,
                            num_elems=STOT8, d=1, num_idxs=NJ)
        TP2 = pp.tile([128, 128], F32, tag="TP2")
        nc.tensor.transpose(TP2[:], G2[:], cid)
        T2r = pw.tile([128, 128], F32, tag="T2r")
        nc.vector.tensor_copy(T2r[:], TP2[:])
        t2m = pw.tile([128, 128], F32, tag="t2m")
        nc.vector.tensor_tensor(
            t2m[:].rearrange("p (c u) -> p c u", u=USPLIT),
            T2r[:].rearrange("p (c u) -> p c u", u=USPLIT),
            ohu[:].unsqueeze(1).to_broadcast([128, 16, USPLIT]), ALU.mult)
        nc.vector.reduce_sum(T[:, 128:C],
                             t2m[:].rearrange("p (c u) -> p c u", u=USPLIT),
                             axis=AX.X)
        # layer 0 chunks
        for k in range(NCH):
            tpc = gather_tr(FaL[0][:, k * CHS:(k + 1) * CHS], sidxC[k],
                            CHS, f"c{k}")
            dst = T[:, 0:128] if k == NCH - 1 else selA[:]
            nc.vector.scalar_tensor_tensor(dst, tpc[:], PK[:, k:k + 1],
                                           selA[:], ALU.mult, ALU.add)

        if mode == "gather":
            osb = pw.tile([2, 1], F32, tag="osb")
            nc.vector.tensor_copy(osb[:], T[0:2, 0:1])
            nc.sync.dma_start(io["out"], osb[:])
            return

        S = pw.tile([128, 2], F32, tag="S")

        # ---- focal classification loss ----
        z = T[:, 64:C]  # [128, 80] logits
        ez = pw.tile([128, N_CLS], F32, tag="ez")
        lnin = pw.tile([128, 5], F32, tag="lnin")  # [sez | se4] jointly ln'd
        sez = lnin[:, 0:1]
        i_expz = nc.scalar.activation(ez[:], z, ACT.Exp, accum_out=sez)
        add_dep_helper(i_expz.ins, i_warm.ins, sync=False,
                       reason="reuse pre-warmed Exp table")
        d64 = T[:, 0:64]
        ed = pw.tile([128, 64], F32, tag="ed")
        i_expd = nc.scalar.activation(ed[:], d64, ACT.Exp)
        # logit select depends only on T -> runs on DVE while ACT exps
        zm = pw.tile([128, N_CLS], F32, tag="zm")
        nc.vector.tensor_mul(zm[:], z, oh[:])
        zsel = pw.tile([128, 1], F32, tag="zsel")
        nc.vector.reduce_sum(zsel[:], zm[:], axis=AX.X)
        nc.vector.reduce_sum(lnin[:, 1:5],
                             ed[:].rearrange("p (a b) -> p a b", b=N_BINS),
                             axis=AX.X)
        # ln on DVE (exponent/mantissa split + quartic) -- avoids the 1.3us
        # ACT Ln table load on the critical tail. |err| < 1.5e-4.
        xi = lnin[:].bitcast(I32)
        ei = pw.tile([128, 5], I32, tag="ei")
        nc.vector.tensor_scalar(ei[:], xi, 23, -127,
                                ALU.logical_shift_right, ALU.add)
        ef = pw.tile([128, 5], F32, tag="ef")
        nc.vector.tensor_copy(ef[:], ei[:])
        mi = pw.tile([128, 5], I32, tag="mi")
        nc.vector.tensor_scalar(mi[:], xi, 0x7FFFFF, 0x3F800000,
                                ALU.bitwise_and, ALU.bitwise_or)
        mf = mi[:].bitcast(F32)
        lp5 = pw.tile([128, 5], F32, tag="lp5")
        nc.vector.tensor_scalar(lp5[:], mf, 0.10668473, -0.71359,
                                ALU.mult, ALU.add)
        for cc in (2.08687922, -1.47904892):
            nc.vector.tensor_tensor(lp5[:], lp5[:], mf, ALU.mult)
            nc.vector.tensor_scalar(lp5[:], lp5[:], cc, None, ALU.add)
        ln5 = pw.tile([128, 5], F32, tag="ln5")
        nc.vector.scalar_tensor_tensor(ln5[:], ef[:], 0.6931471805599453,
                                       lp5[:], ALU.mult, ALU.add)
        lse = ln5[:, 0:1]
        lse4 = ln5[:, 1:5]

        ce = pw.tile([128, 1], F32, tag="ce")
        nc.vector.tensor_sub(ce[:], lse, zsel[:])
        # pt = exp(-ce) = exp(zsel)/sez; the exp rides the warm ACT table
        esel = pw.tile([128, 1], F32, tag="esel")
        nc.scalar.activation(esel[:], zsel[:], ACT.Exp)
        pt = pw.tile([128, 1], F32, tag="pt")
        nc.vector.tensor_tensor(pt[:], esel[:], sez, ALU.divide)
        u1 = pw.tile([128, 1], F32, tag="u1")
        nc.vector.tensor_scalar(u1[:], pt[:], -1.0, 1.0, ALU.mult, ALU.add)
        u2 = pw.tile([128, 1], F32, tag="u2")
        nc.vector.tensor_mul(u2[:], u1[:], u1[:])
        nc.vector.tensor_mul(u2[:], u2[:], ce[:])
        nc.vector.tensor_mul(S[:, 0:1], u2[:], cvalid)

        # ---- DFL box loss ----
        def pick(ohx, tag):
            dm = pw.tile([128, 64], F32, tag=tag + "_dm")
            nc.vector.tensor_mul(dm[:], d64, ohx[:])
            dsel = pw.tile([128, 4], F32, tag=tag + "_d")
            nc.vector.reduce_sum(dsel[:],
                                 dm[:].rearrange("p (a b) -> p a b", b=N_BINS),
                                 axis=AX.X)
            return dsel

        dl = pick(ohl, "dl")
        dr = pick(ohr, "dr")
        # (dl-lse4)*wl + (dr-lse4)*wr == dl*wl + dr*wr - lse4  (wl+wr == 1)
        lpl = pw.tile([128, 4], F32, tag="lpl")
        nc.vector.tensor_mul(lpl[:], dl[:], wl[:])
        lpr = pw.tile([128, 4], F32, tag="lpr")
        nc.vector.tensor_mul(lpr[:], dr[:], wr[:])
        acc = pw.tile([128, 4], F32, tag="acc")
        nc.vector.tensor_add(acc[:], lpl[:], lpr[:])
        nc.vector.tensor_sub(acc[:], acc[:], lse4)
        boxt = pw.tile([128, 1], F32, tag="boxt")
        nc.vector.reduce_sum(boxt[:], acc[:], axis=AX.X)
        nc.vector.tensor_mul(S[:, 1:2], boxt[:], cvneg)

        # ---- reduce the 128 per-target contributions to 2 scalars ----
        PS = pp.tile([2, 1], F32, tag="PS")
        nc.tensor.matmul(PS[:], S[:], cones, start=True, stop=True)
        osb = pw.tile([2, 1], F32, tag="osb")
        nc.vector.tensor_copy(osb[:], PS[:])
        nc.sync.dma_start(io["out"], osb[:])


_CACHE = {}


def _build(reps=1, mode="full"):
    key = f"nc{reps}_{mode}"
    if key in _CACHE:
        return _CACHE[key], _CACHE[key + "_names"]
    nc = bacc.Bacc("TRN2", target_bir_lowering=False, debug=False,
                   enable_asserts=False, num_devices=N_CORES)
    io = {}

    def din(name, shape, dt=F32):
        io[name] = nc.dram_tensor(name, shape, dt, kind="ExternalInput").ap()

    din("feat0", [BPC, C, S0])
    din("feat1", [BPC, C, S1])
    din("feat2", [BPC, C, S2])
    din("cpack", [128, CP_W])
    io["out"] = nc.dram_tensor("out", [2, 1], F32, kind="ExternalOutput").ap()

    with tile.TileContext(nc) as tc:
        with tc.tile_pool(name="feat", bufs=1) as pf, \
             tc.tile_pool(name="wk", bufs=1) as pw, \
             tc.tile_pool(name="ps", bufs=1, space="PSUM") as pp:
            for r in range(reps):
                if r:
                    # isolate repetitions (timing builds only; reps=1 in prod)
                    tc.strict_bb_all_engine_barrier()
                _emit(nc, tc, io, (pf, pw, pp), mode=mode)
    nc.compile()
    _CACHE[key] = nc
    _CACHE[key + "_names"] = list(io)
    return nc, list(io)


def _const_block():
    if "cblk" in _CACHE:
        return _CACHE["cblk"]
    j = np.arange(NJ)
    cb = ((np.arange(8)[None, :] * 16 + (j[:, None] % 16)) // NT_PAD)
    out = {
        "cid": np.eye(128, dtype=np.float32),
        "ciota": np.broadcast_to(np.arange(N_CLS, dtype=np.float32),
                                 (128, N_CLS)).copy(),
        "cones": np.ones((128, 1), np.float32),
        "cvalid": ((j % NT_PAD) < N_TGT).astype(np.float32)[:, None],
        "cvneg": -((j % NT_PAD) < N_TGT).astype(np.float32)[:, None],
        "cbt": (j // NT_PAD).astype(np.float32)[:, None],
        "chw3": np.broadcast_to(
            np.array([WS[0] / 2, WS[1] / 2, WS[2] / 2], np.float32),
            (128, 3)).copy(),
        "cinv3": np.broadcast_to(
            np.array([8 / S0, 8 / S1, 8 / S2], np.float32), (128, 3)).copy(),
        "cb": cb.astype(np.float32),
    }
    _CACHE["cblk"] = out
    return out


def _per_core_inputs(feat0, feat1, feat2, targets, core):
    b0 = core * BPC
    tpad = np.zeros((BPC, NT_PAD, 6), np.float32)
    tpad[:, :, 5] = 3.0  # pad rows match no layer
    tpad[:, :N_TGT, :] = targets[b0:b0 + BPC]
    tpad = tpad.reshape(NJ, 6)

    # wrapped+replicated layout: w[p, col] = field[col*16 + p%16]
    wi = (np.arange(8)[None, :] * 16 + (np.arange(128)[:, None] % 16))
    twr = np.concatenate([tpad[:, 1][wi], tpad[:, 2][wi], tpad[:, 5][wi]],
                         axis=1).astype(np.float32)

    cb = _const_block()
    cpack = np.empty((128, CP_W), np.float32)
    cpack[:, CP_ID:CP_ID + 128] = cb["cid"]
    cpack[:, CP_IOTA:CP_IOTA + N_CLS] = cb["ciota"]
    cpack[:, CP_ONES:CP_ONES + 1] = cb["cones"]
    cpack[:, CP_VALID:CP_VALID + 1] = cb["cvalid"]
    cpack[:, CP_VNEG:CP_VNEG + 1] = cb["cvneg"]
    cpack[:, CP_CBT:CP_CBT + 1] = cb["cbt"]
    cpack[:, CP_HW3:CP_HW3 + 3] = cb["chw3"]
    cpack[:, CP_INV3:CP_INV3 + 3] = cb["cinv3"]
    cpack[:, CP_B:CP_B + 8] = cb["cb"]
    cpack[:, CP_TWR:CP_TWR + 24] = twr
    cpack[:, CP_TGT:CP_TGT + 6] = tpad

    return {
        "feat0": np.ascontiguousarray(feat0[b0:b0 + BPC].reshape(BPC, C, S0)),
        "feat1": np.ascontiguousarray(feat1[b0:b0 + BPC].reshape(BPC, C, S1)),
        "feat2": np.ascontiguousarray(feat2[b0:b0 + BPC].reshape(BPC, C, S2)),
        "cpack": cpack,
    }


def kernel(feat0, feat1, feat2, targets):
    nc, _ = _build()
    in_maps = [_per_core_inputs(feat0, feat1, feat2, targets, k)
               for k in range(N_CORES)]
    res = run_bass_kernel_spmd(nc, in_maps, core_ids=list(range(N_CORES)))
    parts = np.stack([r["out"].reshape(2) for r in res.results])  # [8, 2]
    cls_sum = np.float32(parts[:, 0].sum(dtype=np.float32))
    box_sum = np.float32(parts[:, 1].sum(dtype=np.float32))
    total = np.float32(cls_sum + box_sum)
    return (total, cls_sum, box_sum)
